# revision 15
# baseline (speedup 1.0000x reference)
"""Multi-head attention (B=8, N=1024, C=1024, H=16) on 8 TRN2 NeuronCores.

Strategy: pure data parallelism — each core computes one batch element with
replicated weights (no collectives). Per-core single-head-dim layout:

  inputs (host-prepped, transposed so every matmul contracts on partitions):
    xT   [C, N]      = x[b].T
    wqkT [C, 2C_qk]  = in_proj_weight[0:2048].T     (q then k features)
    wvT  [C, C]      = in_proj_weight[2048:3072].T
    woT  [C, C]      = out_proj_weight.T            ((h,d) rows, co cols)
  phases on-device (all matmuls in float32r: full-rate fp32, ~1e-3 rounding):
    A: V natural [token, vfeat] per 65-wide head group w/ ones column
       (the ones column makes the PV matmul also produce softmax row-sums)
    B: qkT [feature, token] (transposed q/k for QK^T)
    C: per head h: S^T[key, query] = K_h^T.T @ Q_h^T; P = exp(S*scale);
       O^T[d+1, query] = V_hat.T @ P^T (row 64 = softmax sums);
       normalize via reciprocal + gpsimd partition_broadcast
    D: outT [co, token] = woT.T @ A^T + bias

Output: outT per core, host transposes back and stacks.
"""
import numpy as np

B, N, C = 8, 1024, 1024
H = 16
HD = C // H               # 64
SCALE = HD ** (-0.5)
NCORES = 8

_COMPILED = {}


def _build():
    import concourse.bass as bass
    import concourse.tile as tile
    from concourse import bacc, mybir

    F32 = mybir.dt.float32
    F32R = mybir.dt.float32r
    EXP = mybir.ActivationFunctionType.Exp

    nc = bacc.Bacc("TRN2", target_bir_lowering=False, debug=False)

    xT = nc.dram_tensor("xT", [C, N], F32R, kind="ExternalInput").ap()
    wqkT = nc.dram_tensor("wqkT", [C, 2 * C], F32R, kind="ExternalInput").ap()
    wvT = nc.dram_tensor("wvT", [C, C], F32R, kind="ExternalInput").ap()
    woT = nc.dram_tensor("woT", [C, C], F32R, kind="ExternalInput").ap()
    bqk = nc.dram_tensor("bqk", [128, 16], F32, kind="ExternalInput").ap()
    bv = nc.dram_tensor("bv", [1, C], F32R, kind="ExternalInput").ap()
    bo = nc.dram_tensor("bo", [128, 8], F32, kind="ExternalInput").ap()
    ones_col = nc.dram_tensor("ones_col", [128, 16], F32R, kind="ExternalInput").ap()
    ones_row = nc.dram_tensor("ones_row", [1, 512], F32R, kind="ExternalInput").ap()
    outT = nc.dram_tensor("outT", [C, N], F32, kind="ExternalOutput").ap()

    CB = C // 128      # 8 contraction blocks
    TB = N // 128      # 8 token blocks
    JB = 2 * C // 128  # 16 qk feature blocks
    VW = 65            # per-head V width (64 feats + ones col)

    with tile.TileContext(nc) as tc:
        with tc.tile_pool(name="misc", bufs=1) as pool_misc, \
             tc.tile_pool(name="V", bufs=1) as pool_V, \
             tc.tile_pool(name="qk", bufs=1) as pool_qk:

            bqk_sb = pool_misc.tile([128, 16], F32, tag="bqk")
            bv_sb = pool_misc.tile([1, C], F32R, tag="bv")
            bo_sb = pool_misc.tile([128, 8], F32, tag="bo")
            ones_sb = pool_misc.tile([1, 512], F32R, tag="ones")
            nc.sync.dma_start(bqk_sb[:, :], bqk)
            nc.sync.dma_start(bv_sb[:, :], bv)
            nc.sync.dma_start(bo_sb[:, :], bo)
            nc.sync.dma_start(ones_sb[:, :], ones_row)

            V_sb = [pool_V.tile([128, H * VW], F32R, tag=f"V{tb}", name=f"V{tb}") for tb in range(TB)]
            qk_sb = [pool_qk.tile([128, N], F32R, tag=f"qk{jb}", name=f"qk{jb}") for jb in range(JB)]

            # ======== phases A (V natural) and B (qkT) ========
            with tc.tile_pool(name="x", bufs=1) as pool_x, \
                 tc.tile_pool(name="ps_proj", bufs=4, space="PSUM") as ps_proj:

                x_sb = [pool_x.tile([128, N], F32R, tag=f"x{cb}", name=f"x{cb}") for cb in range(CB)]

                # ---- A: V[token, vfeat] ----
                with tc.tile_pool(name="wv", bufs=1) as pool_wv:
                    wv_sb = [pool_wv.tile([128, C], F32R, tag=f"wv{cb}", name=f"wv{cb}") for cb in range(CB)]
                    # load order: x fully first (both A and B need it), then wv
                    for cb in range(CB):
                        for ch in range(2):
                            nc.sync.dma_start(
                                x_sb[cb][:, ch * 512:(ch + 1) * 512],
                                xT[cb * 128:(cb + 1) * 128, ch * 512:(ch + 1) * 512])
                    for cb in range(CB):
                        for ch in range(2):
                            nc.sync.dma_start(
                                wv_sb[cb][:, ch * 512:(ch + 1) * 512],
                                wvT[cb * 128:(cb + 1) * 128, ch * 512:(ch + 1) * 512])
                    # ones columns of V_hat groups (only needed by phase C's PV)
                    for tb in range(TB):
                        nc.sync.dma_start(V_sb[tb][:, 64::VW], ones_col)
                    for tb in range(TB):
                        for vc in range(2):
                            ps = ps_proj.tile([128, 512], F32, tag="psA")
                            for cb in range(CB):
                                nc.tensor.matmul(
                                    ps[:, :],
                                    x_sb[cb][:, tb * 128:(tb + 1) * 128],
                                    wv_sb[cb][:, vc * 512:(vc + 1) * 512],
                                    start=(cb == 0), stop=False,
                                )
                            nc.tensor.matmul(
                                ps[:, :],
                                ones_sb[0:1, 0:128],
                                bv_sb[0:1, vc * 512:(vc + 1) * 512],
                                start=False, stop=True,
                            )
                            # scatter 8 heads x 64 cols into the 65-strided layout
                            dst = V_sb[tb][:, vc * 8 * VW:(vc + 1) * 8 * VW]
                            dst3 = dst.rearrange("p (h d) -> p h d", h=8)[:, :, 0:64]
                            src3 = ps[:, :].rearrange("p (h d) -> p h d", h=8)
                            nc.vector.tensor_copy(dst3, src3)

                # ---- B: qkT[feature, token] ----
                with tc.tile_pool(name="wqk", bufs=10) as pool_wqk:
                    # k-feature half (jh=1) first so attention pairs can start
                    # as soon as their q block lands in the second half
                    for jh in (1, 0):  # stream wqk in two 1024-feature halves
                        wqk_sb = []
                        for cb in range(CB):
                            t = pool_wqk.tile([128, C], F32R, tag="wqk", name="wqk")
                            nc.sync.dma_start(
                                t[:, :],
                                wqkT[cb * 128:(cb + 1) * 128, jh * C:(jh + 1) * C],
                            )
                            wqk_sb.append(t)
                        for jbl in range(8):
                            jb = jh * 8 + jbl
                            for nch in range(2):
                                ps = ps_proj.tile([128, 512], F32, tag="psA")
                                for cb in range(CB):
                                    nc.tensor.matmul(
                                        ps[:, :],
                                        wqk_sb[cb][:, jbl * 128:(jbl + 1) * 128],
                                        x_sb[cb][:, nch * 512:(nch + 1) * 512],
                                        start=(cb == 0), stop=(cb == CB - 1),
                                    )
                                nc.vector.tensor_scalar(
                                    qk_sb[jb][:, nch * 512:(nch + 1) * 512], ps[:, :],
                                    bqk_sb[:, jb:jb + 1], None, mybir.AluOpType.add,
                                )

            # ======== phases C (attention) and D (out projection) ========
            # A^T reuses the q-feature qk tiles: block hp's q/k data is dead
            # once pair hp's S^T matmuls are done.
            A_sb = qk_sb[0:8]
            with tc.tile_pool(name="wo", bufs=1) as pool_wo:
                wo_sb = [pool_wo.tile([128, C], F32R, tag=f"wo{cb}", name=f"wo{cb}") for cb in range(CB)]
                for cb in range(CB):
                    nc.sync.dma_start(wo_sb[cb][:, :], woT[cb * 128:(cb + 1) * 128, :])

                with tc.tile_pool(name="PT", bufs=6) as pool_PT, \
                     tc.tile_pool(name="norm", bufs=2) as pool_norm, \
                     tc.tile_pool(name="ps_S", bufs=2, space="PSUM") as ps_S, \
                     tc.tile_pool(name="ps_O", bufs=2, space="PSUM") as ps_O:

                    for hp in range(8):
                        qb, kb_blk = hp, 8 + hp  # qkT block indices for this pair
                        o_ps = {hh: ps_O.tile([VW, N], F32, tag="O", name="O")
                                for hh in range(2)}
                        # interleaved: S^T -> exp -> PV per key block; the two
                        # heads of the pair land on different PE row groups
                        for kb in range(TB):
                            for hh in range(2):
                                h = 2 * hp + hh
                                r0, r1 = hh * 64, hh * 64 + 64
                                s_ps = ps_S.tile([128, N], F32, tag="S")
                                for ic in range(2):
                                    nc.tensor.matmul(
                                        s_ps[:, ic * 512:(ic + 1) * 512],
                                        qk_sb[kb_blk][r0:r1, kb * 128:(kb + 1) * 128],
                                        qk_sb[qb][r0:r1, ic * 512:(ic + 1) * 512],
                                        start=True, stop=True,
                                    )
                                p_t = pool_PT.tile([128, N], F32R, tag="pt")
                                nc.scalar.activation(p_t[:, :], s_ps[:, :], EXP,
                                                     scale=float(SCALE))
                                for ic in range(2):
                                    nc.tensor.matmul(
                                        o_ps[hh][:, ic * 512:(ic + 1) * 512],
                                        V_sb[kb][:, h * VW:(h + 1) * VW],
                                        p_t[:, ic * 512:(ic + 1) * 512],
                                        start=(kb == 0), stop=(kb == TB - 1),
                                    )
                        # normalize per head
                        for hh in range(2):
                            h = 2 * hp + hh
                            # copy O psum -> sbuf immediately (releases the
                            # psum bank for the next pair's PV accumulation)
                            o_cp = pool_norm.tile([VW, N], F32, tag="ocp")
                            nc.vector.tensor_copy(o_cp[:, :], o_ps[hh][:, :])
                            # sums row 64 -> [128, 8] (fast multi-lane recip),
                            # back to [1, N] at partition 0, broadcast to 64
                            s128 = pool_norm.tile([128, 8], F32, tag="s128")
                            nc.sync.dma_start(s128[:, :], o_cp[64:65, :])
                            r128 = pool_norm.tile([128, 8], F32, tag="r128")
                            nc.vector.reciprocal(r128[:, :], s128[:, :])
                            r0t = pool_norm.tile([1, N], F32, tag="r0")
                            nc.sync.dma_start(r0t[0:1, :], r128[:, :])
                            r_rep = pool_norm.tile([64, N], F32, tag="rrep")
                            nc.gpsimd.partition_broadcast(r_rep[:, :], r0t[0:1, :])
                            if hh == 0:
                                nc.vector.tensor_mul(
                                    A_sb[hp][0:64, :], o_cp[0:64, :], r_rep[:, :])
                            else:
                                a_tmp = pool_norm.tile([64, N], F32R, tag="atmp")
                                nc.vector.tensor_mul(
                                    a_tmp[:, :], o_cp[0:64, :], r_rep[:, :])
                                nc.sync.dma_start(A_sb[hp][64:128, :], a_tmp[:, :])

                with tc.tile_pool(name="outp", bufs=3) as pool_out, \
                     tc.tile_pool(name="ps_out", bufs=4, space="PSUM") as ps_out:
                    for cb in range(CB):
                        for nch in range(2):
                            ps = ps_out.tile([128, 512], F32, tag="po")
                            for hb in range(8):
                                nc.tensor.matmul(
                                    ps[:, :],
                                    wo_sb[hb][:, cb * 128:(cb + 1) * 128],
                                    A_sb[hb][:, nch * 512:(nch + 1) * 512],
                                    start=(hb == 0), stop=(hb == 7),
                                )
                            o_t = pool_out.tile([128, 512], F32, tag="ot")
                            nc.vector.tensor_scalar(
                                o_t[:, :], ps[:, :], bo_sb[:, cb:cb + 1], None,
                                mybir.AluOpType.add,
                            )
                            nc.sync.dma_start(
                                outT[cb * 128:(cb + 1) * 128,
                                     nch * 512:(nch + 1) * 512],
                                o_t[:, :],
                            )
    nc.compile()
    return nc


def _get_nc():
    if "nc" not in _COMPILED:
        _COMPILED["nc"] = _build()
    return _COMPILED["nc"]


def _run(x, in_proj_weight, in_proj_bias, out_proj_weight, out_proj_bias,
         trace=False):
    from concourse.bass_utils import run_bass_kernel_spmd

    nc = _get_nc()
    x = np.ascontiguousarray(np.asarray(x, dtype=np.float32))
    w_in = np.asarray(in_proj_weight, dtype=np.float32)
    b_in = np.asarray(in_proj_bias, dtype=np.float32)
    w_out = np.asarray(out_proj_weight, dtype=np.float32)
    b_out = np.asarray(out_proj_bias, dtype=np.float32)

    wqkT = np.ascontiguousarray(w_in[0:2 * C].T)          # [C, 2C]
    wvT = np.ascontiguousarray(w_in[2 * C:3 * C].T)       # [C, C]
    woT = np.ascontiguousarray(w_out.T)                   # [C, C]
    shared = {
        "wqkT": wqkT,
        "wvT": wvT,
        "woT": woT,
        "bqk": np.ascontiguousarray(b_in[0:2 * C].reshape(16, 128).T),
        "bv": np.ascontiguousarray(b_in[2 * C:3 * C])[None, :],
        "bo": np.ascontiguousarray(b_out.reshape(8, 128).T),
        "ones_col": np.ones((128, 16), dtype=np.float32),
        "ones_row": np.ones((1, 512), dtype=np.float32),
    }
    in_maps = []
    for c in range(NCORES):
        m = dict(shared)
        m["xT"] = np.ascontiguousarray(x[c].T)
        in_maps.append(m)

    res = run_bass_kernel_spmd(nc, in_maps, core_ids=list(range(NCORES)),
                               trace=trace)
    out = np.stack([
        np.ascontiguousarray(res.results[c]["outT"].T) for c in range(NCORES)
    ]).astype(np.float32)
    return out, res


def kernel(x, in_proj_weight, in_proj_bias, out_proj_weight, out_proj_bias):
    out, _ = _run(x, in_proj_weight, in_proj_bias, out_proj_weight,
                  out_proj_bias)
    return out


# revision 16
# speedup vs baseline: 1.1386x; 1.1386x over previous
"""Multi-head attention (B=8, N=1024, C=1024, H=16) on 8 TRN2 NeuronCores.

Strategy: pure data parallelism — each core computes one batch element with
replicated weights (no collectives). Per-core single-head-dim layout:

  inputs (host-prepped, transposed so every matmul contracts on partitions):
    xT   [C, N]      = x[b].T
    wqkT [C, 2C_qk]  = in_proj_weight[0:2048].T     (q then k features)
    wvT  [C, C]      = in_proj_weight[2048:3072].T
    woT  [C, C]      = out_proj_weight.T            ((h,d) rows, co cols)
  phases on-device (all matmuls in float32r: full-rate fp32, ~1e-3 rounding):
    A: V natural [token, vfeat] per 65-wide head group w/ ones column
       (the ones column makes the PV matmul also produce softmax row-sums)
    B: qkT [feature, token] (transposed q/k for QK^T)
    C: per head h: S^T[key, query] = K_h^T.T @ Q_h^T; P = exp(S*scale);
       O^T[d+1, query] = V_hat.T @ P^T (row 64 = softmax sums);
       normalize via reciprocal + gpsimd partition_broadcast
    D: outT [co, token] = woT.T @ A^T + bias

Output: outT per core, host transposes back and stacks.
"""
import numpy as np

B, N, C = 8, 1024, 1024
H = 16
HD = C // H               # 64
SCALE = HD ** (-0.5)
NCORES = 8

_COMPILED = {}


def _build():
    import concourse.bass as bass
    import concourse.tile as tile
    from concourse import bacc, mybir

    F32 = mybir.dt.float32
    F32R = mybir.dt.float32r
    EXP = mybir.ActivationFunctionType.Exp

    nc = bacc.Bacc("TRN2", target_bir_lowering=False, debug=False)

    xT = nc.dram_tensor("xT", [C, N], F32R, kind="ExternalInput").ap()
    wqkT = nc.dram_tensor("wqkT", [C, 2 * C], F32R, kind="ExternalInput").ap()
    wvT = nc.dram_tensor("wvT", [C, C], F32R, kind="ExternalInput").ap()
    woT = nc.dram_tensor("woT", [C, C], F32R, kind="ExternalInput").ap()
    bqk = nc.dram_tensor("bqk", [128, 16], F32, kind="ExternalInput").ap()
    bv = nc.dram_tensor("bv", [1, C], F32R, kind="ExternalInput").ap()
    bo = nc.dram_tensor("bo", [128, 8], F32, kind="ExternalInput").ap()
    ones_col = nc.dram_tensor("ones_col", [128, 16], F32R, kind="ExternalInput").ap()
    ones_row = nc.dram_tensor("ones_row", [1, 512], F32R, kind="ExternalInput").ap()
    outT = nc.dram_tensor("outT", [C, N], F32, kind="ExternalOutput").ap()

    CB = C // 128      # 8 contraction blocks
    TB = N // 128      # 8 token blocks
    JB = 2 * C // 128  # 16 qk feature blocks
    VW = 65            # per-head V width (64 feats + ones col)

    with tile.TileContext(nc) as tc:
        with tc.tile_pool(name="misc", bufs=1) as pool_misc, \
             tc.tile_pool(name="V", bufs=1) as pool_V, \
             tc.tile_pool(name="qk", bufs=1) as pool_qk:

            bqk_sb = pool_misc.tile([128, 16], F32, tag="bqk")
            bv_sb = pool_misc.tile([1, C], F32R, tag="bv")
            bo_sb = pool_misc.tile([128, 8], F32, tag="bo")
            ones_sb = pool_misc.tile([1, 512], F32R, tag="ones")
            nc.sync.dma_start(bqk_sb[:, :], bqk)
            nc.sync.dma_start(bv_sb[:, :], bv)
            nc.sync.dma_start(bo_sb[:, :], bo)
            nc.sync.dma_start(ones_sb[:, :], ones_row)

            V_sb = [pool_V.tile([128, H * VW], F32R, tag=f"V{tb}", name=f"V{tb}") for tb in range(TB)]
            qk_sb = [pool_qk.tile([128, N], F32R, tag=f"qk{jb}", name=f"qk{jb}") for jb in range(JB)]

            # ======== phases A (V natural) and B (qkT) ========
            with tc.tile_pool(name="x", bufs=1) as pool_x, \
                 tc.tile_pool(name="ps_proj", bufs=4, space="PSUM") as ps_proj:

                x_sb = [pool_x.tile([128, N], F32R, tag=f"x{cb}", name=f"x{cb}") for cb in range(CB)]

                # ---- A: V[token, vfeat] ----
                with tc.tile_pool(name="wv", bufs=1) as pool_wv:
                    wv_sb = [pool_wv.tile([128, C], F32R, tag=f"wv{cb}", name=f"wv{cb}") for cb in range(CB)]
                    # load order: x fully first (both A and B need it), then wv
                    for cb in range(CB):
                        for ch in range(2):
                            nc.sync.dma_start(
                                x_sb[cb][:, ch * 512:(ch + 1) * 512],
                                xT[cb * 128:(cb + 1) * 128, ch * 512:(ch + 1) * 512])
                    for cb in range(CB):
                        for ch in range(2):
                            nc.sync.dma_start(
                                wv_sb[cb][:, ch * 512:(ch + 1) * 512],
                                wvT[cb * 128:(cb + 1) * 128, ch * 512:(ch + 1) * 512])
                    # ones columns of V_hat groups (only needed by phase C's PV)
                    for tb in range(TB):
                        nc.sync.dma_start(V_sb[tb][:, 64::VW], ones_col)
                    for tb in range(TB):
                        for vc in range(2):
                            ps = ps_proj.tile([128, 512], F32, tag="psA")
                            for cb in range(CB):
                                nc.tensor.matmul(
                                    ps[:, :],
                                    x_sb[cb][:, tb * 128:(tb + 1) * 128],
                                    wv_sb[cb][:, vc * 512:(vc + 1) * 512],
                                    start=(cb == 0), stop=False,
                                )
                            nc.tensor.matmul(
                                ps[:, :],
                                ones_sb[0:1, 0:128],
                                bv_sb[0:1, vc * 512:(vc + 1) * 512],
                                start=False, stop=True,
                            )
                            # scatter 8 heads x 64 cols into the 65-strided layout
                            dst = V_sb[tb][:, vc * 8 * VW:(vc + 1) * 8 * VW]
                            dst3 = dst.rearrange("p (h d) -> p h d", h=8)[:, :, 0:64]
                            src3 = ps[:, :].rearrange("p (h d) -> p h d", h=8)
                            nc.vector.tensor_copy(dst3, src3)

                # ---- B: qkT[feature, token] ----
                with tc.tile_pool(name="wqk", bufs=10) as pool_wqk:
                    # k-feature half (jh=1) first so attention pairs can start
                    # as soon as their q block lands in the second half
                    for jh in (1, 0):  # stream wqk in two 1024-feature halves
                        wqk_sb = []
                        for cb in range(CB):
                            t = pool_wqk.tile([128, C], F32R, tag="wqk", name="wqk")
                            nc.sync.dma_start(
                                t[:, :],
                                wqkT[cb * 128:(cb + 1) * 128, jh * C:(jh + 1) * C],
                            )
                            wqk_sb.append(t)
                        for jbl in range(8):
                            jb = jh * 8 + jbl
                            for nch in range(2):
                                ps = ps_proj.tile([128, 512], F32, tag="psA")
                                for cb in range(CB):
                                    nc.tensor.matmul(
                                        ps[:, :],
                                        wqk_sb[cb][:, jbl * 128:(jbl + 1) * 128],
                                        x_sb[cb][:, nch * 512:(nch + 1) * 512],
                                        start=(cb == 0), stop=(cb == CB - 1),
                                    )
                                nc.vector.tensor_scalar(
                                    qk_sb[jb][:, nch * 512:(nch + 1) * 512], ps[:, :],
                                    bqk_sb[:, jb:jb + 1], None, mybir.AluOpType.add,
                                )

            # ======== phases C (attention) and D (out projection) ========
            # A^T reuses the q-feature qk tiles: block hp's q/k data is dead
            # once pair hp's S^T matmuls are done.
            A_sb = qk_sb[0:8]
            with tc.tile_pool(name="wo", bufs=1) as pool_wo:
                wo_sb = [pool_wo.tile([128, C], F32R, tag=f"wo{cb}", name=f"wo{cb}") for cb in range(CB)]
                for cb in range(CB):
                    nc.sync.dma_start(wo_sb[cb][:, :], woT[cb * 128:(cb + 1) * 128, :])

                with tc.tile_pool(name="PT", bufs=6) as pool_PT, \
                     tc.tile_pool(name="norm", bufs=2) as pool_norm, \
                     tc.tile_pool(name="ps_S", bufs=2, space="PSUM") as ps_S, \
                     tc.tile_pool(name="ps_O", bufs=2, space="PSUM") as ps_O:

                    for hp in range(8):
                        qb, kb_blk = hp, 8 + hp  # qkT block indices for this pair
                        o_ps = {hh: ps_O.tile([VW, N], F32, tag="O", name="O")
                                for hh in range(2)}
                        # interleaved: S^T -> exp -> PV per key block; the two
                        # heads of the pair land on different PE row groups
                        for kb in range(TB):
                            for hh in range(2):
                                h = 2 * hp + hh
                                r0, r1 = hh * 64, hh * 64 + 64
                                s_ps = ps_S.tile([128, N], F32, tag="S")
                                for ic in range(2):
                                    nc.tensor.matmul(
                                        s_ps[:, ic * 512:(ic + 1) * 512],
                                        qk_sb[kb_blk][r0:r1, kb * 128:(kb + 1) * 128],
                                        qk_sb[qb][r0:r1, ic * 512:(ic + 1) * 512],
                                        start=True, stop=True,
                                    )
                                p_t = pool_PT.tile([128, N], F32R, tag="pt")
                                nc.scalar.activation(p_t[:, :], s_ps[:, :], EXP,
                                                     scale=float(SCALE))
                                for ic in range(2):
                                    nc.tensor.matmul(
                                        o_ps[hh][:, ic * 512:(ic + 1) * 512],
                                        V_sb[kb][:, h * VW:(h + 1) * VW],
                                        p_t[:, ic * 512:(ic + 1) * 512],
                                        start=(kb == 0), stop=(kb == TB - 1),
                                    )
                        # normalize per head
                        for hh in range(2):
                            h = 2 * hp + hh
                            # sums row 64 -> [128, 8] (fast multi-lane recip),
                            # back to [1, N] at partition 0, broadcast to 64
                            s_hi = pool_norm.tile([VW, N], F32, tag="shi")
                            nc.vector.tensor_copy(s_hi[64:65, :], o_ps[hh][64:65, :])
                            s128 = pool_norm.tile([128, 8], F32, tag="s128")
                            nc.sync.dma_start(s128[:, :], s_hi[64:65, :])
                            r128 = pool_norm.tile([128, 8], F32, tag="r128")
                            nc.vector.reciprocal(r128[:, :], s128[:, :])
                            r0t = pool_norm.tile([1, N], F32, tag="r0")
                            nc.sync.dma_start(r0t[0:1, :], r128[:, :])
                            r_rep = pool_norm.tile([64, N], F32, tag="rrep")
                            nc.gpsimd.partition_broadcast(r_rep[:, :], r0t[0:1, :])
                            if hh == 0:
                                nc.vector.tensor_mul(
                                    A_sb[hp][0:64, :], o_ps[hh][0:64, :], r_rep[:, :])
                            else:
                                a_tmp = pool_norm.tile([64, N], F32R, tag="atmp")
                                nc.vector.tensor_mul(
                                    a_tmp[:, :], o_ps[hh][0:64, :], r_rep[:, :])
                                nc.sync.dma_start(A_sb[hp][64:128, :], a_tmp[:, :])

                with tc.tile_pool(name="outp", bufs=3) as pool_out, \
                     tc.tile_pool(name="ps_out", bufs=4, space="PSUM") as ps_out:
                    for cb in range(CB):
                        for nch in range(2):
                            ps = ps_out.tile([128, 512], F32, tag="po")
                            for hb in range(8):
                                nc.tensor.matmul(
                                    ps[:, :],
                                    wo_sb[hb][:, cb * 128:(cb + 1) * 128],
                                    A_sb[hb][:, nch * 512:(nch + 1) * 512],
                                    start=(hb == 0), stop=(hb == 7),
                                )
                            o_t = pool_out.tile([128, 512], F32, tag="ot")
                            nc.vector.tensor_scalar(
                                o_t[:, :], ps[:, :], bo_sb[:, cb:cb + 1], None,
                                mybir.AluOpType.add,
                            )
                            nc.sync.dma_start(
                                outT[cb * 128:(cb + 1) * 128,
                                     nch * 512:(nch + 1) * 512],
                                o_t[:, :],
                            )
    nc.compile()
    return nc


def _get_nc():
    if "nc" not in _COMPILED:
        _COMPILED["nc"] = _build()
    return _COMPILED["nc"]


def _run(x, in_proj_weight, in_proj_bias, out_proj_weight, out_proj_bias,
         trace=False):
    from concourse.bass_utils import run_bass_kernel_spmd

    nc = _get_nc()
    x = np.ascontiguousarray(np.asarray(x, dtype=np.float32))
    w_in = np.asarray(in_proj_weight, dtype=np.float32)
    b_in = np.asarray(in_proj_bias, dtype=np.float32)
    w_out = np.asarray(out_proj_weight, dtype=np.float32)
    b_out = np.asarray(out_proj_bias, dtype=np.float32)

    wqkT = np.ascontiguousarray(w_in[0:2 * C].T)          # [C, 2C]
    wvT = np.ascontiguousarray(w_in[2 * C:3 * C].T)       # [C, C]
    woT = np.ascontiguousarray(w_out.T)                   # [C, C]
    shared = {
        "wqkT": wqkT,
        "wvT": wvT,
        "woT": woT,
        "bqk": np.ascontiguousarray(b_in[0:2 * C].reshape(16, 128).T),
        "bv": np.ascontiguousarray(b_in[2 * C:3 * C])[None, :],
        "bo": np.ascontiguousarray(b_out.reshape(8, 128).T),
        "ones_col": np.ones((128, 16), dtype=np.float32),
        "ones_row": np.ones((1, 512), dtype=np.float32),
    }
    in_maps = []
    for c in range(NCORES):
        m = dict(shared)
        m["xT"] = np.ascontiguousarray(x[c].T)
        in_maps.append(m)

    res = run_bass_kernel_spmd(nc, in_maps, core_ids=list(range(NCORES)),
                               trace=trace)
    out = np.stack([
        np.ascontiguousarray(res.results[c]["outT"].T) for c in range(NCORES)
    ]).astype(np.float32)
    return out, res


def kernel(x, in_proj_weight, in_proj_bias, out_proj_weight, out_proj_bias):
    out, _ = _run(x, in_proj_weight, in_proj_bias, out_proj_weight,
                  out_proj_bias)
    return out
